# revision 1
# baseline (speedup 1.0000x reference)
"""GCNConv (graph message passing) on 8 Trainium2 NeuronCores — Bass/Tile.

out = a + (a @ Wres + bres),  a = relu(segment_sum(edge_val * (xW+b)[edge_col],
edge_row)),  computed via the identity  agg_lin = (A@x) @ W + deg x b  so the
sparse part runs on raw x.

Sharding: nodes (segment-sum destinations) are partitioned across the 8 cores
(12500 nodes each); x and the small dense weights are replicated; each core
processes exactly the edges whose destination lands in its shard (host-side
routing — the "route messages for cross-partition edges" step of the hint).

Per-core device algorithm (fully transposed, features on partitions):
  Phase 1: for each 128-destination block, gather the needed source rows with
  dma_gather (int16 indices into <=25000-row chunks of x) and accumulate
  psum[f, d] += xg_tile.T @ S over the block's 128-edge tiles, where
  S[e, d] = onehot(dest_in_block(e)) * edge_val(e) is built on the vector
  engine from an iota constant with a single two-op tensor_scalar
  (is_equal then mult).  PSUM accumulates across all source chunks of a
  block, then flushes to an SBUF aggT [128 features, 12544 dests].
  Phase 2 (slabs of 4 blocks): psA = W.T @ aggT_slab + b x deg (rank-1
  matmul), aT = relu(psA); psB = Wres.T @ aT + bres x 1; outT = psB + aT,
  stored transposed [128, 12544] per core; the host transposes + concatenates.
"""
import math
import numpy as np

import concourse.tile as tile
from concourse import bacc, mybir
from concourse.bass_utils import run_bass_kernel_spmd

F32 = mybir.dt.float32
I16 = mybir.dt.int16
AL = mybir.AluOpType
D = 128
P = 128
N_CORES = 8
CH = 25000        # x chunk rows (int16 gather indices => <= 32767)
SBW = 4           # destination blocks per superblock (one gather per chunk)


def _build(n_src, n_blocks, tpb, repeat=1):
    nsh_pad = n_blocks * P
    Q = math.ceil(n_src / CH)
    G = Q * n_blocks * tpb
    IC = G * 8
    sbs = [list(range(s, min(s + SBW, n_blocks))) for s in range(0, n_blocks, SBW)]

    nc = bacc.Bacc("TRN2", target_bir_lowering=False, debug=False)
    x = nc.dram_tensor("x", [n_src, D], F32, kind="ExternalInput")
    W = nc.dram_tensor("W", [D, D], F32, kind="ExternalInput")
    Wres = nc.dram_tensor("Wres", [D, D], F32, kind="ExternalInput")
    bvec = nc.dram_tensor("bvec", [1, D], F32, kind="ExternalInput")
    bres = nc.dram_tensor("bres", [1, D], F32, kind="ExternalInput")
    iotaf = nc.dram_tensor("iotaf", [P, P], F32, kind="ExternalInput")
    idx = nc.dram_tensor("idx", [P, IC], I16, kind="ExternalInput")
    darr = nc.dram_tensor("darr", [P, G], F32, kind="ExternalInput")
    varr = nc.dram_tensor("varr", [P, G], F32, kind="ExternalInput")
    deg = nc.dram_tensor("deg", [1, nsh_pad], F32, kind="ExternalInput")
    outT = nc.dram_tensor("outT", [D, nsh_pad], F32, kind="ExternalOutput")

    with tile.TileContext(nc) as tc:
        with tc.tile_pool(name="const", bufs=1) as cp:
            W_sb = cp.tile([D, D], F32)
            nc.sync.dma_start(W_sb[:], W.ap())
            Wres_sb = cp.tile([D, D], F32)
            nc.sync.dma_start(Wres_sb[:], Wres.ap())
            b_sb = cp.tile([1, D], F32)
            nc.sync.dma_start(b_sb[:], bvec.ap())
            bres_sb = cp.tile([1, D], F32)
            nc.sync.dma_start(bres_sb[:], bres.ap())
            deg_sb = cp.tile([1, nsh_pad], F32)
            nc.sync.dma_start(deg_sb[:], deg.ap())
            iota_f = cp.tile([P, P], F32)
            nc.sync.dma_start(iota_f[:], iotaf.ap())
            idx_sb = cp.tile([P, IC], I16)
            nc.sync.dma_start(idx_sb[:], idx.ap())
            d_sb = cp.tile([P, G], F32)
            nc.sync.dma_start(d_sb[:], darr.ap())
            v_sb = cp.tile([P, G], F32)
            nc.sync.dma_start(v_sb[:], varr.ap())
            ones_row = cp.tile([1, 512], F32)
            nc.vector.memset(ones_row[:], 1.0)
            aggT = cp.tile([D, nsh_pad], F32)

            for _rep in range(repeat):
                # ---- Phase 1: gather + one-hot-matmul segment sum ----
                with (
                    tc.tile_pool(name="xg", bufs=Q + 1) as xg_pool,
                    tc.tile_pool(name="s", bufs=6) as s_pool,
                    tc.tile_pool(name="ps1", bufs=4, space="PSUM") as ps1,
                ):
                    gt = 0
                    cbase = 0
                    for sb in sbs:
                        nb = len(sb)
                        nidx = nb * tpb * P
                        xgs = []
                        for q in range(Q):
                            xg = xg_pool.tile([P, nb * tpb * P], F32, tag="xg",
                                              name=f"xg{q}")
                            nc.gpsimd.dma_gather(
                                xg[:].rearrange("p (t f) -> p t f", f=P),
                                x.ap()[q * CH: min(n_src, (q + 1) * CH), :],
                                idx_sb[:, cbase: cbase + nidx // 16],
                                nidx, nidx, D,
                                single_packet=(nidx <= 1024),
                            )
                            cbase += nidx // 16
                            xgs.append(xg)
                        pss = [ps1.tile([P, P], F32, tag="ps", name=f"ps{j}")
                               for j in range(nb)]
                        for q in range(Q):
                            for j in range(nb):
                                for t in range(tpb):
                                    S = s_pool.tile([P, P], F32, name="S")
                                    nc.vector.tensor_scalar(
                                        S[:], iota_f[:],
                                        d_sb[:, gt:gt + 1], v_sb[:, gt:gt + 1],
                                        op0=AL.is_equal, op1=AL.mult,
                                    )
                                    e0 = (j * tpb + t) * P
                                    nc.tensor.matmul(
                                        out=pss[j][:],
                                        lhsT=xgs[q][:, e0:e0 + P],
                                        rhs=S[:],
                                        start=(q == 0 and t == 0),
                                        stop=(q == Q - 1 and t == tpb - 1),
                                    )
                                    gt += 1
                        for j, k in enumerate(sb):
                            nc.vector.tensor_copy(aggT[:, k * P:(k + 1) * P],
                                                  pss[j][:])

                # ---- Phase 2: dense head ----
                SLAB = 4 * P
                with (
                    tc.tile_pool(name="a", bufs=2) as a_pool,
                    tc.tile_pool(name="o", bufs=2) as o_pool,
                    tc.tile_pool(name="psA", bufs=2, space="PSUM") as psA_pool,
                    tc.tile_pool(name="psB", bufs=2, space="PSUM") as psB_pool,
                ):
                    for s0 in range(0, nsh_pad, SLAB):
                        w = min(SLAB, nsh_pad - s0)
                        psA = psA_pool.tile([P, SLAB], F32)
                        nc.tensor.matmul(out=psA[:, :w], lhsT=W_sb[:],
                                         rhs=aggT[:, s0:s0 + w],
                                         start=True, stop=False)
                        nc.tensor.matmul(out=psA[:, :w], lhsT=b_sb[:1, :],
                                         rhs=deg_sb[:1, s0:s0 + w],
                                         start=False, stop=True)
                        a_t = a_pool.tile([P, SLAB], F32)
                        nc.scalar.activation(a_t[:, :w], psA[:, :w],
                                             mybir.ActivationFunctionType.Relu)
                        psB = psB_pool.tile([P, SLAB], F32)
                        nc.tensor.matmul(out=psB[:, :w], lhsT=Wres_sb[:],
                                         rhs=a_t[:, :w], start=True, stop=False)
                        nc.tensor.matmul(out=psB[:, :w], lhsT=bres_sb[:1, :],
                                         rhs=ones_row[:1, :w],
                                         start=False, stop=True)
                        o_t = o_pool.tile([P, SLAB], F32)
                        nc.vector.tensor_tensor(o_t[:, :w], psB[:, :w],
                                                a_t[:, :w], op=AL.add)
                        nc.sync.dma_start(outT.ap()[:, s0:s0 + w], o_t[:, :w])

    nc.compile()
    return nc


def _prep(x, W, b, Wres, bres, edge_val, edge_row, edge_col):
    x = np.ascontiguousarray(np.asarray(x, np.float32))
    W = np.ascontiguousarray(np.asarray(W, np.float32))
    Wres = np.ascontiguousarray(np.asarray(Wres, np.float32))
    b = np.asarray(b, np.float32).reshape(1, D)
    bres_v = np.asarray(bres, np.float32).reshape(1, D)
    edge_row = np.asarray(edge_row)
    edge_col = np.asarray(edge_col)
    edge_val = np.asarray(edge_val, np.float32)

    N = x.shape[0]
    Q = math.ceil(N / CH)
    nsh = math.ceil(N / N_CORES)
    n_blocks = math.ceil(nsh / P)
    nsh_pad = n_blocks * P
    n_groups = n_blocks * Q

    shards = []
    tpb = 1
    for c in range(N_CORES):
        lo = c * nsh
        hi = min(N, lo + nsh)
        m = (edge_row >= lo) & (edge_row < hi)
        r = (edge_row[m] - lo).astype(np.int64)
        ci = edge_col[m].astype(np.int64)
        v = edge_val[m]
        blk = r >> 7
        q = ci // CH
        counts = np.bincount(blk * Q + q, minlength=n_groups)
        tpb = max(tpb, int(math.ceil(counts.max() / P)))
        shards.append((r, ci, v, blk, q))

    G = Q * n_blocks * tpb
    IC = G * 8
    sbs = [list(range(s, min(s + SBW, n_blocks))) for s in range(0, n_blocks, SBW)]
    grp_tile0 = np.zeros((n_blocks, Q), np.int64)
    tcur = 0
    for sb in sbs:
        nb = len(sb)
        for q in range(Q):
            for j, k in enumerate(sb):
                grp_tile0[k, q] = tcur + j * tpb
            tcur += nb * tpb
    assert tcur == G

    iota_f = np.tile(np.arange(P, dtype=np.float32), (P, 1))

    in_maps = []
    for c in range(N_CORES):
        r, ci, v, blk, q = shards[c]
        gid = blk * Q + q
        order = np.argsort(gid, kind="stable")
        r, ci, v, blk, q, gid = (a[order] for a in (r, ci, v, blk, q, gid))
        starts = np.zeros(n_groups + 1, np.int64)
        np.cumsum(np.bincount(gid, minlength=n_groups), out=starts[1:])
        ranks = np.arange(len(r), dtype=np.int64) - starts[gid]
        slot = (grp_tile0[blk, q] + (ranks >> 7)) * P + (ranks & 127)

        idx16 = np.zeros(G * P, np.int16)
        d_flat = np.zeros(G * P, np.float32)
        v_flat = np.zeros(G * P, np.float32)
        idx16[slot] = (ci - q * CH).astype(np.int16)
        d_flat[slot] = (r & 127).astype(np.float32)
        v_flat[slot] = v
        idx_h = np.tile(np.ascontiguousarray(idx16.reshape(IC, 16).T), (8, 1))
        d_h = np.ascontiguousarray(d_flat.reshape(G, P).T)
        v_h = np.ascontiguousarray(v_flat.reshape(G, P).T)
        degv = np.zeros(nsh_pad, np.float32)
        degv[:nsh] += np.bincount(r, weights=v, minlength=nsh
                                  ).astype(np.float32)[:nsh]
        in_maps.append({
            "x": x, "W": W, "Wres": Wres, "bvec": b, "bres": bres_v,
            "iotaf": iota_f, "idx": idx_h, "darr": d_h, "varr": v_h,
            "deg": degv.reshape(1, nsh_pad),
        })
    meta = dict(N=N, nsh=nsh, n_blocks=n_blocks, nsh_pad=nsh_pad, tpb=tpb, Q=Q)
    return in_maps, meta


def kernel(x, W, b, Wres, bres, edge_val, edge_row, edge_col):
    in_maps, meta = _prep(x, W, b, Wres, bres, edge_val, edge_row, edge_col)
    nc = _build(np.asarray(x).shape[0], meta["n_blocks"], meta["tpb"])
    res = run_bass_kernel_spmd(nc, in_maps, core_ids=list(range(N_CORES)))
    N, nsh = meta["N"], meta["nsh"]
    out = np.empty((N, D), np.float32)
    for c in range(N_CORES):
        lo = c * nsh
        hi = min(N, lo + nsh)
        out[lo:hi] = res.results[c]["outT"].T[: hi - lo]
    return out



# revision 7
# speedup vs baseline: 410.0277x; 410.0277x over previous
"""GCNConv (graph message passing) on 8 Trainium2 NeuronCores — Bass/Tile.

out = a + (a @ Wres + bres),  a = relu(segment_sum(edge_val * (xW+b)[edge_col],
edge_row)),  computed via the identity  agg_lin = (A@x) @ W + deg x b  so the
sparse part runs on raw x.

Sharding: nodes (segment-sum destinations) are partitioned across the 8 cores
(12500 nodes each); x and the small dense weights are replicated; each core
processes exactly the edges whose destination lands in its shard (host-side
routing — the "route messages for cross-partition edges" step of the hint).

Per-core device algorithm (fully transposed, features on partitions):
  Phase 1: for each 128-destination block, gather the needed source rows in
  bf16 with dma_gather (int16 indices into <=25000-row chunks of x) and
  accumulate psum[f, d] += xg_tile.T @ S over the block's 128-edge tiles,
  where S[e, d] = onehot(dest_in_block(e)) * edge_val(e) is built in bf16 on
  the vector engine from an iota constant with a single two-op tensor_scalar
  (is_equal then mult).  PSUM accumulates across all source chunks of a
  block, then flushes to an SBUF aggT [128 features, 12544 dests] (f32).
  Phase 2 (slabs of 4 blocks, fp32r matmuls): psA = W.T @ aggT_slab +
  b x deg (rank-1), aT = relu(psA); psB = Wres.T @ aT + bres x 1;
  outT = psB + aT, stored transposed [128, 12544] per core; the host
  transposes + concatenates.
"""
import math
import numpy as np
import ml_dtypes

import concourse.tile as tile
from concourse import bacc, mybir
from concourse.bass_utils import run_bass_kernel_spmd

F32 = mybir.dt.float32
F32R = mybir.dt.float32r
BF16 = mybir.dt.bfloat16
I16 = mybir.dt.int16
AL = mybir.AluOpType
D = 128
P = 128
N_CORES = 8
CH = 25000        # x chunk rows (int16 gather indices => <= 32767)
SBW = 4           # destination blocks per superblock (one gather per chunk)


def _build(n_src, n_blocks, tpb, repeat=1):
    nsh_pad = n_blocks * P
    Q = math.ceil(n_src / CH)
    G = Q * n_blocks * tpb
    IC = G * 8
    sbs = [list(range(s, min(s + SBW, n_blocks))) for s in range(0, n_blocks, SBW)]

    nc = bacc.Bacc("TRN2", target_bir_lowering=False, debug=False)
    x = nc.dram_tensor("x", [n_src, D], BF16, kind="ExternalInput")
    W = nc.dram_tensor("W", [D, D], BF16, kind="ExternalInput")
    Wres = nc.dram_tensor("Wres", [D, D], BF16, kind="ExternalInput")
    bvec = nc.dram_tensor("bvec", [1, D], BF16, kind="ExternalInput")
    bres = nc.dram_tensor("bres", [1, D], BF16, kind="ExternalInput")
    iotaf = nc.dram_tensor("iotaf", [P, P], BF16, kind="ExternalInput")
    idx = nc.dram_tensor("idx", [P, IC], I16, kind="ExternalInput")
    darr = nc.dram_tensor("darr", [P, G], F32, kind="ExternalInput")
    varr = nc.dram_tensor("varr", [P, G], F32, kind="ExternalInput")
    deg = nc.dram_tensor("deg", [1, nsh_pad], BF16, kind="ExternalInput")
    outT = nc.dram_tensor("outT", [D, nsh_pad], F32, kind="ExternalOutput")

    with tile.TileContext(nc) as tc:
        with tc.tile_pool(name="const", bufs=1) as cp:
            W_sb = cp.tile([D, D], BF16)
            nc.sync.dma_start(W_sb[:], W.ap())
            Wres_sb = cp.tile([D, D], BF16)
            nc.sync.dma_start(Wres_sb[:], Wres.ap())
            b_sb = cp.tile([1, D], BF16)
            nc.sync.dma_start(b_sb[:], bvec.ap())
            bres_sb = cp.tile([1, D], BF16)
            nc.sync.dma_start(bres_sb[:], bres.ap())
            deg_sb = cp.tile([1, nsh_pad], BF16)
            nc.sync.dma_start(deg_sb[:], deg.ap())
            iota_f = cp.tile([P, P], BF16)
            nc.sync.dma_start(iota_f[:], iotaf.ap())
            idx_sb = cp.tile([P, IC], I16)
            nc.sync.dma_start(idx_sb[:], idx.ap())
            d_sb = cp.tile([P, G], F32)
            nc.sync.dma_start(d_sb[:], darr.ap())
            v_sb = cp.tile([P, G], F32)
            nc.sync.dma_start(v_sb[:], varr.ap())
            ones_row = cp.tile([1, 512], BF16)
            nc.vector.memset(ones_row[:], 1.0)
            aggT = cp.tile([D, nsh_pad], BF16)

            for _rep in range(repeat):
                # ---- Phase 1: gather + one-hot-matmul segment sum ----
                SLAB = 4 * P
                with (
                    tc.tile_pool(name="xg", bufs=2 * Q) as xg_pool,
                    tc.tile_pool(name="s", bufs=8) as s_pool,
                    tc.tile_pool(name="ps1", bufs=2, space="PSUM") as ps1,
                    tc.tile_pool(name="a", bufs=2) as a_pool,
                    tc.tile_pool(name="o", bufs=2) as o_pool,
                    tc.tile_pool(name="psA", bufs=2, space="PSUM") as psA_pool,
                    tc.tile_pool(name="psB", bufs=2, space="PSUM") as psB_pool,
                ):
                    def dense_head(s0):
                        w = min(SLAB, nsh_pad - s0)
                        psA = psA_pool.tile([P, SLAB], F32, name="psA")
                        nc.tensor.matmul(out=psA[:, :w],
                                         lhsT=W_sb[:],
                                         rhs=aggT[:, s0:s0 + w],
                                         start=True, stop=False)
                        nc.tensor.matmul(out=psA[:, :w],
                                         lhsT=b_sb[:1, :],
                                         rhs=deg_sb[:1, s0:s0 + w],
                                         start=False, stop=True)
                        a_t = a_pool.tile([P, SLAB], BF16, name="a_t")
                        nc.scalar.activation(a_t[:, :w], psA[:, :w],
                                             mybir.ActivationFunctionType.Relu)
                        psB = psB_pool.tile([P, SLAB], F32, name="psB")
                        nc.tensor.matmul(out=psB[:, :w],
                                         lhsT=Wres_sb[:],
                                         rhs=a_t[:, :w],
                                         start=True, stop=False)
                        nc.tensor.matmul(out=psB[:, :w],
                                         lhsT=bres_sb[:1, :],
                                         rhs=ones_row[:1, :w],
                                         start=False, stop=True)
                        o_t = o_pool.tile([P, SLAB], F32, name="o_t")
                        nc.vector.tensor_tensor(o_t[:, :w], psB[:, :w],
                                                a_t[:, :w], op=AL.add)
                        nc.sync.dma_start(outT.ap()[:, s0:s0 + w], o_t[:, :w])

                    gt = 0
                    cbase = 0
                    flushed = 0
                    for sb in sbs:
                        nb = len(sb)
                        nidx = nb * tpb * P
                        xgs = []
                        for q in range(Q):
                            xg = xg_pool.tile([P, nb * tpb * P], BF16, tag="xg",
                                              name=f"xg{q}")
                            nc.gpsimd.dma_gather(
                                xg[:].rearrange("p (t f) -> p t f", f=P),
                                x.ap()[q * CH: min(n_src, (q + 1) * CH), :],
                                idx_sb[:, cbase: cbase + nidx // 16],
                                nidx, nidx, D,
                                single_packet=(nidx <= 1024),
                            )
                            cbase += nidx // 16
                            xgs.append(xg)
                        pss = ps1.tile([P, SBW * P], F32, tag="ps", name="pss")
                        for q in range(Q):
                            for j in range(nb):
                                for t in range(tpb):
                                    S = s_pool.tile([P, P], BF16, name="S")
                                    nc.vector.tensor_scalar(
                                        S[:], iota_f[:],
                                        d_sb[:, gt:gt + 1], v_sb[:, gt:gt + 1],
                                        op0=AL.is_equal, op1=AL.mult,
                                    )
                                    e0 = (j * tpb + t) * P
                                    nc.tensor.matmul(
                                        out=pss[:, j * P:(j + 1) * P],
                                        lhsT=xgs[q][:, e0:e0 + P],
                                        rhs=S[:],
                                        start=(q == 0 and j == 0 and t == 0),
                                        stop=(q == Q - 1 and j == nb - 1
                                              and t == tpb - 1),
                                    )
                                    gt += 1
                        k0 = sb[0]
                        nc.vector.tensor_copy(aggT[:, k0 * P:(k0 + nb) * P],
                                              pss[:, :nb * P])
                        # ---- Phase 2 (dense head), interleaved per slab ----
                        done = sb[-1] + 1
                        while flushed + SLAB // P <= done:
                            dense_head(flushed * P)
                            flushed += SLAB // P
                    while flushed * P < nsh_pad:
                        dense_head(flushed * P)
                        flushed += SLAB // P

    nc.compile()
    return nc


def _prep(x, W, b, Wres, bres, edge_val, edge_row, edge_col):
    x = np.ascontiguousarray(np.asarray(x, np.float32)).astype(ml_dtypes.bfloat16)
    W = np.ascontiguousarray(np.asarray(W, np.float32)).astype(ml_dtypes.bfloat16)
    Wres = np.ascontiguousarray(np.asarray(Wres, np.float32)).astype(ml_dtypes.bfloat16)
    b = np.asarray(b, np.float32).reshape(1, D).astype(ml_dtypes.bfloat16)
    bres_v = np.asarray(bres, np.float32).reshape(1, D).astype(ml_dtypes.bfloat16)
    edge_row = np.asarray(edge_row)
    edge_col = np.asarray(edge_col)
    edge_val = np.asarray(edge_val, np.float32)

    N = x.shape[0]
    Q = math.ceil(N / CH)
    nsh = math.ceil(N / N_CORES)
    n_blocks = math.ceil(nsh / P)
    nsh_pad = n_blocks * P
    n_groups = n_blocks * Q

    shards = []
    tpb = 1
    for c in range(N_CORES):
        lo = c * nsh
        hi = min(N, lo + nsh)
        m = (edge_row >= lo) & (edge_row < hi)
        r = (edge_row[m] - lo).astype(np.int64)
        ci = edge_col[m].astype(np.int64)
        v = edge_val[m]
        blk = r >> 7
        q = ci // CH
        counts = np.bincount(blk * Q + q, minlength=n_groups)
        tpb = max(tpb, int(math.ceil(counts.max() / P)))
        shards.append((r, ci, v, blk, q))

    G = Q * n_blocks * tpb
    IC = G * 8
    sbs = [list(range(s, min(s + SBW, n_blocks))) for s in range(0, n_blocks, SBW)]
    grp_tile0 = np.zeros((n_blocks, Q), np.int64)
    tcur = 0
    for sb in sbs:
        nb = len(sb)
        for q in range(Q):
            for j, k in enumerate(sb):
                grp_tile0[k, q] = tcur + j * tpb
            tcur += nb * tpb
    assert tcur == G

    iota_f = np.tile(np.arange(P, dtype=np.float32),
                     (P, 1)).astype(ml_dtypes.bfloat16)

    in_maps = []
    for c in range(N_CORES):
        r, ci, v, blk, q = shards[c]
        gid = blk * Q + q
        order = np.argsort(gid, kind="stable")
        r, ci, v, blk, q, gid = (a[order] for a in (r, ci, v, blk, q, gid))
        starts = np.zeros(n_groups + 1, np.int64)
        np.cumsum(np.bincount(gid, minlength=n_groups), out=starts[1:])
        ranks = np.arange(len(r), dtype=np.int64) - starts[gid]
        slot = (grp_tile0[blk, q] + (ranks >> 7)) * P + (ranks & 127)

        idx16 = np.zeros(G * P, np.int16)
        d_flat = np.zeros(G * P, np.float32)
        v_flat = np.zeros(G * P, np.float32)
        idx16[slot] = (ci - q * CH).astype(np.int16)
        d_flat[slot] = (r & 127).astype(np.float32)
        v_flat[slot] = v
        idx_h = np.tile(np.ascontiguousarray(idx16.reshape(IC, 16).T), (8, 1))
        d_h = np.ascontiguousarray(d_flat.reshape(G, P).T)
        v_h = np.ascontiguousarray(v_flat.reshape(G, P).T)
        degv = np.zeros(nsh_pad, np.float32)
        degv[:nsh] += np.bincount(r, weights=v, minlength=nsh
                                  ).astype(np.float32)[:nsh]
        in_maps.append({
            "x": x, "W": W, "Wres": Wres, "bvec": b, "bres": bres_v,
            "iotaf": iota_f, "idx": idx_h, "darr": d_h, "varr": v_h,
            "deg": degv.reshape(1, nsh_pad).astype(ml_dtypes.bfloat16),
        })
    meta = dict(N=N, nsh=nsh, n_blocks=n_blocks, nsh_pad=nsh_pad, tpb=tpb, Q=Q)
    return in_maps, meta


def kernel(x, W, b, Wres, bres, edge_val, edge_row, edge_col):
    in_maps, meta = _prep(x, W, b, Wres, bres, edge_val, edge_row, edge_col)
    nc = _build(np.asarray(x).shape[0], meta["n_blocks"], meta["tpb"])
    res = run_bass_kernel_spmd(nc, in_maps, core_ids=list(range(N_CORES)))
    N, nsh = meta["N"], meta["nsh"]
    out = np.empty((N, D), np.float32)
    for c in range(N_CORES):
        lo = c * nsh
        hi = min(N, lo + nsh)
        out[lo:hi] = res.results[c]["outT"].T[: hi - lo]
    return out


# revision 8
# speedup vs baseline: 464.1571x; 1.1320x over previous
"""GCNConv on 8 TRN2 NeuronCores — v3: fp16 + shared-overflow tiles.

Same dest-sharded one-hot-matmul segment sum as v2, but the per-(block,chunk)
tile count is capped at 4 (512 edges); excess edges go to n_ov shared
"overflow" tiles per (superblock, chunk) whose one-hot spans the whole
superblock (width nb*128 <= 512).  This removes most gather padding
(250880 -> ~230400 slots/core).  Everything in phase 1/2 is fp16 (exact
integers to 2048 for the wide one-hot iota; better mantissa than bf16);
PSUM stays f32.
"""
import math
import numpy as np

import concourse.tile as tile
from concourse import bacc, mybir
from concourse.bass_utils import run_bass_kernel_spmd

F32 = mybir.dt.float32
FP16 = mybir.dt.float16
I16 = mybir.dt.int16
AL = mybir.AluOpType
D = 128
P = 128
N_CORES = 8
CH = 25000        # x chunk rows (int16 gather indices => <= 32767)
SBW = 4           # destination blocks per superblock / psum bank
NREG = 4          # regular tiles per (block, chunk): cap 512 edges


def _sb_list(n_blocks):
    return [list(range(s, min(s + SBW, n_blocks)))
            for s in range(0, n_blocks, SBW)]


def _tiles_per_sbq(nb, n_ov):
    return NREG * nb + n_ov


def _build(n_src, n_blocks, n_ov, repeat=1):
    nsh_pad = n_blocks * P
    Q = math.ceil(n_src / CH)
    sbs = _sb_list(n_blocks)
    G = sum(Q * _tiles_per_sbq(len(sb), n_ov) for sb in sbs)
    IC = G * 8

    nc = bacc.Bacc("TRN2", target_bir_lowering=False, debug=False,
                   num_swdge_queues=4)
    x = nc.dram_tensor("x", [n_src, D], FP16, kind="ExternalInput")
    W = nc.dram_tensor("W", [D, D], FP16, kind="ExternalInput")
    Wres = nc.dram_tensor("Wres", [D, D], FP16, kind="ExternalInput")
    bvec = nc.dram_tensor("bvec", [1, D], FP16, kind="ExternalInput")
    bres = nc.dram_tensor("bres", [1, D], FP16, kind="ExternalInput")
    iotaf = nc.dram_tensor("iotaf", [P, SBW * P], FP16, kind="ExternalInput")
    idx = nc.dram_tensor("idx", [P, IC], I16, kind="ExternalInput")
    darr = nc.dram_tensor("darr", [P, G], F32, kind="ExternalInput")
    varr = nc.dram_tensor("varr", [P, G], F32, kind="ExternalInput")
    deg = nc.dram_tensor("deg", [1, nsh_pad], FP16, kind="ExternalInput")
    outT = nc.dram_tensor("outT", [D, nsh_pad], FP16, kind="ExternalOutput")

    with tile.TileContext(nc) as tc:
        with tc.tile_pool(name="const", bufs=1) as cp:
            W_sb = cp.tile([D, D], FP16)
            nc.sync.dma_start(W_sb[:], W.ap())
            Wres_sb = cp.tile([D, D], FP16)
            nc.sync.dma_start(Wres_sb[:], Wres.ap())
            b_sb = cp.tile([1, D], FP16)
            nc.sync.dma_start(b_sb[:], bvec.ap())
            bres_sb = cp.tile([1, D], FP16)
            nc.sync.dma_start(bres_sb[:], bres.ap())
            deg_sb = cp.tile([1, nsh_pad], FP16)
            nc.sync.dma_start(deg_sb[:], deg.ap())
            iota_f = cp.tile([P, SBW * P], FP16)
            nc.sync.dma_start(iota_f[:], iotaf.ap())
            idx_sb = cp.tile([P, IC], I16)
            nc.sync.dma_start(idx_sb[:], idx.ap())
            d_sb = cp.tile([P, G], F32)
            nc.sync.dma_start(d_sb[:], darr.ap())
            v_sb = cp.tile([P, G], F32)
            nc.sync.dma_start(v_sb[:], varr.ap())
            ones_row = cp.tile([1, 512], FP16)
            nc.vector.memset(ones_row[:], 1.0)
            aggT = cp.tile([D, nsh_pad], FP16)

            for _rep in range(repeat):
                SLAB = 4 * P
                with (
                    tc.tile_pool(name="xg", bufs=2 * Q) as xg_pool,
                    tc.tile_pool(name="s", bufs=8) as s_pool,
                    tc.tile_pool(name="ps1", bufs=2, space="PSUM") as ps1,
                    tc.tile_pool(name="a", bufs=2) as a_pool,
                    tc.tile_pool(name="o", bufs=2) as o_pool,
                    tc.tile_pool(name="psA", bufs=2, space="PSUM") as psA_pool,
                    tc.tile_pool(name="psB", bufs=2, space="PSUM") as psB_pool,
                ):
                    def dense_head(s0):
                        w = min(SLAB, nsh_pad - s0)
                        psA = psA_pool.tile([P, SLAB], F32, name="psA")
                        nc.tensor.matmul(out=psA[:, :w], lhsT=W_sb[:],
                                         rhs=aggT[:, s0:s0 + w],
                                         start=True, stop=False)
                        nc.tensor.matmul(out=psA[:, :w], lhsT=b_sb[:1, :],
                                         rhs=deg_sb[:1, s0:s0 + w],
                                         start=False, stop=True)
                        a_t = a_pool.tile([P, SLAB], FP16, name="a_t")
                        nc.scalar.activation(a_t[:, :w], psA[:, :w],
                                             mybir.ActivationFunctionType.Relu)
                        psB = psB_pool.tile([P, SLAB], F32, name="psB")
                        nc.tensor.matmul(out=psB[:, :w], lhsT=Wres_sb[:],
                                         rhs=a_t[:, :w], start=True, stop=False)
                        nc.tensor.matmul(out=psB[:, :w], lhsT=bres_sb[:1, :],
                                         rhs=ones_row[:1, :w],
                                         start=False, stop=True)
                        o_t = o_pool.tile([P, SLAB], FP16, name="o_t")
                        nc.vector.tensor_tensor(o_t[:, :w], psB[:, :w],
                                                a_t[:, :w], op=AL.add)
                        nc.sync.dma_start(outT.ap()[:, s0:s0 + w], o_t[:, :w])

                    gt = 0
                    cbase = 0
                    flushed = 0
                    for sb in sbs:
                        nb = len(sb)
                        ntile = _tiles_per_sbq(nb, n_ov)
                        nidx = ntile * P
                        xgs = []
                        for q in range(Q):
                            xg = xg_pool.tile([P, ntile * P], FP16, tag="xg",
                                              name=f"xg{q}")
                            nc.gpsimd.dma_gather(
                                xg[:].rearrange("p (t f) -> p t f", f=P),
                                x.ap()[q * CH: min(n_src, (q + 1) * CH), :],
                                idx_sb[:, cbase: cbase + nidx // 16],
                                nidx, nidx, D,
                                queue_num=q % 4,
                                single_packet=(nidx <= 1024),
                            )
                            cbase += nidx // 16
                            xgs.append(xg)
                        pss = ps1.tile([P, SBW * P], F32, tag="ps", name="pss")
                        for q in range(Q):
                            # regular tiles: one-hot within one block
                            for j in range(nb):
                                for t in range(NREG):
                                    S = s_pool.tile([P, P], FP16, name="S",
                                                    tag="s")
                                    nc.vector.tensor_scalar(
                                        S[:], iota_f[:, :P],
                                        d_sb[:, gt:gt + 1], v_sb[:, gt:gt + 1],
                                        op0=AL.is_equal, op1=AL.mult,
                                    )
                                    e0 = (j * NREG + t) * P
                                    nc.tensor.matmul(
                                        out=pss[:, j * P:(j + 1) * P],
                                        lhsT=xgs[q][:, e0:e0 + P],
                                        rhs=S[:],
                                        start=(q == 0 and j == 0 and t == 0),
                                        stop=False,
                                    )
                                    gt += 1
                            # overflow tiles: one-hot across the superblock
                            for t in range(n_ov):
                                So = s_pool.tile([P, SBW * P], FP16,
                                                 name="So", tag="so")
                                nc.vector.tensor_scalar(
                                    So[:, :nb * P], iota_f[:, :nb * P],
                                    d_sb[:, gt:gt + 1], v_sb[:, gt:gt + 1],
                                    op0=AL.is_equal, op1=AL.mult,
                                )
                                e0 = (nb * NREG + t) * P
                                nc.tensor.matmul(
                                    out=pss[:, :nb * P],
                                    lhsT=xgs[q][:, e0:e0 + P],
                                    rhs=So[:, :nb * P],
                                    start=False,
                                    stop=(q == Q - 1 and t == n_ov - 1),
                                )
                                gt += 1
                        k0 = sb[0]
                        nc.vector.tensor_copy(aggT[:, k0 * P:(k0 + nb) * P],
                                              pss[:, :nb * P])
                        done = sb[-1] + 1
                        while flushed + SLAB // P <= done:
                            dense_head(flushed * P)
                            flushed += SLAB // P
                    while flushed * P < nsh_pad:
                        dense_head(flushed * P)
                        flushed += SLAB // P

    nc.compile()
    return nc


def _prep(x, W, b, Wres, bres, edge_val, edge_row, edge_col):
    x = np.ascontiguousarray(np.asarray(x, np.float32)).astype(np.float16)
    W = np.asarray(W, np.float32).astype(np.float16)
    Wres = np.asarray(Wres, np.float32).astype(np.float16)
    b = np.asarray(b, np.float32).reshape(1, D).astype(np.float16)
    bres_v = np.asarray(bres, np.float32).reshape(1, D).astype(np.float16)
    edge_row = np.asarray(edge_row)
    edge_col = np.asarray(edge_col)
    edge_val = np.asarray(edge_val, np.float32)

    N = x.shape[0]
    Q = math.ceil(N / CH)
    nsh = math.ceil(N / N_CORES)
    n_blocks = math.ceil(nsh / P)
    nsh_pad = n_blocks * P
    n_groups = n_blocks * Q
    sbs = _sb_list(n_blocks)
    nsb = len(sbs)
    REGCAP = NREG * P

    # shard edges, compute overflow -> n_ov (global max across cores)
    shards = []
    n_ov = 1
    for c in range(N_CORES):
        lo = c * nsh
        hi = min(N, lo + nsh)
        m = (edge_row >= lo) & (edge_row < hi)
        r = (edge_row[m] - lo).astype(np.int64)
        ci = edge_col[m].astype(np.int64)
        v = edge_val[m]
        blk = r >> 7
        q = ci // CH
        cnt = np.bincount(blk * Q + q, minlength=n_groups).reshape(n_blocks, Q)
        ov = np.maximum(cnt - REGCAP, 0)
        for s, sb in enumerate(sbs):
            ovs = ov[sb[0]:sb[-1] + 1].sum(axis=0)
            n_ov = max(n_ov, int(math.ceil(ovs.max() / P)))
        shards.append((r, ci, v, blk, q))

    G = sum(Q * _tiles_per_sbq(len(sb), n_ov) for sb in sbs)
    IC = G * 8

    # tile bases in emission order: for sb: for q: [regs][ovs]
    sb_of_block = np.zeros(n_blocks, np.int64)
    for s, sb in enumerate(sbs):
        for k in sb:
            sb_of_block[k] = s
    tile_base = np.zeros((nsb, Q), np.int64)  # first tile of (sb, q)
    tcur = 0
    for s, sb in enumerate(sbs):
        ntile = _tiles_per_sbq(len(sb), n_ov)
        for q in range(Q):
            tile_base[s, q] = tcur + q * ntile
        tcur += Q * ntile
    assert tcur == G

    iota = np.tile(np.arange(SBW * P, dtype=np.float32), (P, 1)
                   ).astype(np.float16)

    in_maps = []
    for c in range(N_CORES):
        r, ci, v, blk, q = shards[c]
        gid = blk * Q + q
        order = np.argsort(gid, kind="stable")
        r, ci, v, blk, q, gid = (a[order] for a in (r, ci, v, blk, q, gid))
        starts = np.zeros(n_groups + 1, np.int64)
        np.cumsum(np.bincount(gid, minlength=n_groups), out=starts[1:])
        ranks = np.arange(len(r), dtype=np.int64) - starts[gid]

        s_of = sb_of_block[blk]
        j_local = blk - np.array([sbs[s][0] for s in s_of], np.int64)
        is_reg = ranks < REGCAP

        slot = np.zeros(len(r), np.int64)
        dval = np.zeros(len(r), np.float32)
        # regular edges
        rr = ranks[is_reg]
        slot[is_reg] = ((tile_base[s_of[is_reg], q[is_reg]]
                         + j_local[is_reg] * NREG + (rr >> 7)) * P
                        + (rr & 127))
        dval[is_reg] = (r[is_reg] & 127).astype(np.float32)
        # overflow edges: pooled per (sb, q), in (j, rank) order (already
        # sorted by gid => by (block, q); within group by original order)
        ovm = ~is_reg
        if ovm.any():
            key = s_of[ovm] * Q + q[ovm]
            oorder = np.argsort(key, kind="stable")
            okey = key[oorder]
            ostarts = np.zeros(nsb * Q + 1, np.int64)
            np.cumsum(np.bincount(okey, minlength=nsb * Q), out=ostarts[1:])
            oranks = np.arange(ovm.sum(), dtype=np.int64) - ostarts[okey]
            assert (oranks < n_ov * P).all(), "overflow capacity exceeded"
            ov_idx = np.where(ovm)[0][oorder]
            nb_arr = np.array([len(sbs[s]) for s in (okey // Q)], np.int64)
            slot[ov_idx] = ((tile_base[okey // Q, okey % Q]
                             + nb_arr * NREG + (oranks >> 7)) * P
                            + (oranks & 127))
            dval[ov_idx] = (j_local[ovm][oorder] * P
                            + (r[ovm][oorder] & 127)).astype(np.float32)

        idx16 = np.zeros(G * P, np.int16)
        d_flat = np.zeros(G * P, np.float32)
        v_flat = np.zeros(G * P, np.float32)
        assert len(np.unique(slot)) == len(slot)
        idx16[slot] = (ci - q * CH).astype(np.int16)
        d_flat[slot] = dval
        v_flat[slot] = v
        idx_h = np.tile(np.ascontiguousarray(idx16.reshape(IC, 16).T), (8, 1))
        d_h = np.ascontiguousarray(d_flat.reshape(G, P).T)
        v_h = np.ascontiguousarray(v_flat.reshape(G, P).T)
        degv = np.zeros(nsh_pad, np.float32)
        degv[:nsh] += np.bincount(r, weights=v, minlength=nsh
                                  ).astype(np.float32)[:nsh]
        in_maps.append({
            "x": x, "W": W, "Wres": Wres, "bvec": b, "bres": bres_v,
            "iotaf": iota, "idx": idx_h, "darr": d_h, "varr": v_h,
            "deg": degv.reshape(1, nsh_pad).astype(np.float16),
        })
    meta = dict(N=N, nsh=nsh, n_blocks=n_blocks, nsh_pad=nsh_pad,
                tpb=n_ov, Q=Q)
    return in_maps, meta


def kernel(x, W, b, Wres, bres, edge_val, edge_row, edge_col):
    in_maps, meta = _prep(x, W, b, Wres, bres, edge_val, edge_row, edge_col)
    nc = _build(np.asarray(x).shape[0], meta["n_blocks"], meta["tpb"])
    res = run_bass_kernel_spmd(nc, in_maps, core_ids=list(range(N_CORES)))
    N, nsh = meta["N"], meta["nsh"]
    out = np.empty((N, D), np.float32)
    for c in range(N_CORES):
        lo = c * nsh
        hi = min(N, lo + nsh)
        out[lo:hi] = res.results[c]["outT"].astype(np.float32).T[: hi - lo]
    return out
